# revision 11
# baseline (speedup 1.0000x reference)
"""GCN2 (GCNII) aggregation + update kernel for 8 Trainium2 NeuronCores.

Sharding: nodes are assigned to cores by striding the global degree-sorted
order (core c gets ranks c, c+8, ...), so every core sees a near-identical
degree profile and one compiled schedule serves all 8 cores with minimal
padding.  Edges are partitioned by destination; per-edge source rows are
halo-materialized host-side in destination-schedule order (bf16) so the
device streams them sequentially at full DMA bandwidth instead of doing
random 256B gathers.

Within a core, paired destination positions (adjacent degree-sorted ranks)
share each 128-lane slot: the slot's stationary operand is [128 lanes, 128]
with the A-instance features in columns 0:64 and the B-instance features in
columns 64:128, so the full 128x128 PE stationary is used.  The moving
operand interleaves one weighted 0/1 column per instance (A at even, B at
odd columns); output rows 0:64 of even columns carry the A aggregate and
rows 64:128 of odd columns the B aggregate (the complementary halves are
ignored garbage).  Per-edge weights deg(src)^-1/2 * deg(dst)^-1/2 * (1-a)
are computed on device from a bf16 degree-product tensor via Ln/Exp;
non-member and pad entries hold 3e38 so their weight underflows to ~0
(5e-20) without any masking ops.  The alpha
residual with x_0 and the (1-beta)I + beta*W1 update run on device.

Host-side work is strictly structural / data rearrangement: appending
self-loops, bincount, sorting, padding, packing, row duplication and dtype
conversion of x.  No floating-point arithmetic is done on the host.
"""
import math
import os
from contextlib import ExitStack

import numpy as np
import ml_dtypes

import concourse.bacc as bacc
import concourse.mybir as mybir
import concourse.tile as tile
from concourse import bass_utils

N_NODES = 65536
C = 64
N_CORES = 8
SHARD = N_NODES // N_CORES          # 8192 dst nodes per core
NPAIR = SHARD // 2                  # 4096 paired positions per core
QBLK = 128                          # positions per psum block
SB_QB = 4                           # q-blocks per superblock
NQB = NPAIR // QBLK                 # 32
NSB = NQB // SB_QB                  # 8 superblocks
ALPHA = 0.1
BETA = math.log(0.5 / 4 + 1.0)

LAST_RESULT = None  # BassKernelResults of the most recent run (for test.py)


# --------------------------------------------------------------------------
# host-side structural prep (no float math)
# --------------------------------------------------------------------------

def _schedule(profile):
    """Greedy slot schedule over paired positions against `profile` (the
    cross-core max of per-pair degree).  Slots never cross a 128-position
    q-block boundary.  Columns are interleaved (A at even, B at odd) and
    numbered locally per superblock."""
    slots = []                       # (q0, M, start, stop, splits, ebase)
    i = 0
    while i < NPAIR:
        dm = int(profile[i])
        if dm > 128:
            q = (dm + 127) // 128
            for j in range(q):
                lanes = min(128, dm - j * 128)
                slots.append((i, 1, j == 0, j == q - 1, [lanes], j * 128))
            i += 1
        else:
            M = 0
            lanes = 0
            splits = []
            while (
                i + M < NPAIR
                and (i % QBLK) + M < QBLK
                and int(profile[i + M]) <= 128 - lanes
            ):
                splits.append(int(profile[i + M]))
                lanes += int(profile[i + M])
                M += 1
            slots.append((i, M, True, True, splits, 0))
            i += M

    ns = len(slots)
    lane_pos = np.full((ns, 128), -1, dtype=np.int64)
    lane_colg = np.full((ns, 128), -1, dtype=np.int64)  # global A-column
    lane_eoff = np.zeros((ns, 128), dtype=np.int64)
    slot_meta = []                   # (q0, M, start, stop, sb, bcol_local)
    sb_ranges = [[None, None] for _ in range(NSB)]
    sb_cols = [0] * NSB
    for si, (q0, M, st, sp, splits, ebase) in enumerate(slots):
        sb = q0 // (QBLK * SB_QB)
        if sb_ranges[sb][0] is None:
            sb_ranges[sb][0] = si
        sb_ranges[sb][1] = si + 1
        bcol = sb_cols[sb]
        lane = 0
        for m, dmx in enumerate(splits):
            lane_pos[si, lane:lane + dmx] = q0 + m
            lane_colg[si, lane:lane + dmx] = bcol + 2 * m  # local; fixed below
            lane_eoff[si, lane:lane + dmx] = ebase + np.arange(dmx)
            lane += dmx
        slot_meta.append((q0, M, st, sp, sb, bcol))
        sb_cols[sb] += 2 * M
    # per-superblock global column bases
    sb_base = np.zeros(NSB + 1, dtype=np.int64)
    np.cumsum(sb_cols, out=sb_base[1:])
    for si, (q0, M, st, sp, sb, bcol) in enumerate(slot_meta):
        mask = lane_colg[si] >= 0
        lane_colg[si, mask] += sb_base[sb]
    SM = int(sb_base[-1])
    sb_col_ranges = [(int(sb_base[s]), int(sb_base[s + 1])) for s in range(NSB)]
    return slot_meta, [tuple(r) for r in sb_ranges], sb_col_ranges, \
        lane_pos, lane_colg, lane_eoff, ns, SM


def _prep(edge_index: np.ndarray):
    src = np.concatenate([edge_index[0], np.arange(N_NODES, dtype=np.int64)])
    dst = np.concatenate([edge_index[1], np.arange(N_NODES, dtype=np.int64)])
    deg = np.bincount(dst, minlength=N_NODES).astype(np.int64)  # incl self-loops
    assert int(deg.max()) ** 2 < 32768

    order = np.argsort(dst, kind="stable")
    src_s = src[order]
    node_start = np.zeros(N_NODES + 1, dtype=np.int64)
    np.cumsum(deg, out=node_start[1:])

    gorder = np.argsort(-deg, kind="stable")       # global degree-sorted nodes
    gdeg = deg[gorder]
    # core c owns gorder[c::8]; pair q = local ranks (2q, 2q+1)
    # profile[q] = max over cores of deg at local rank 2q = gdeg[16q]
    profile = gdeg[0::2 * N_CORES].copy()          # [NPAIR]
    return deg, src_s, node_start, gorder, profile


# --------------------------------------------------------------------------
# device kernel
# --------------------------------------------------------------------------

def _build(ns, SM, slot_meta, sb_ranges, sb_col_ranges):
    f32, bf16, i16 = mybir.dt.float32, mybir.dt.bfloat16, mybir.dt.int16
    nc = bacc.Bacc("TRN2", debug=False, num_devices=N_CORES)

    d_stream = nc.dram_tensor("stream", [128, ns, 128], bf16, kind="ExternalInput")
    d_bp = nc.dram_tensor("bp", [128, SM], bf16, kind="ExternalInput")
    d_x0t = nc.dram_tensor("x0t", [C, SHARD], bf16, kind="ExternalInput")
    d_w1 = nc.dram_tensor("w1", [C, C], f32, kind="ExternalInput")
    d_iden64 = nc.dram_tensor("iden64", [C, C], f32, kind="ExternalInput")
    d_out = nc.dram_tensor("out", [C, SHARD], bf16, kind="ExternalOutput")

    sb_scnt = [hi - lo for lo, hi in sb_ranges]
    sb_cmax = max(hi - lo for lo, hi in sb_col_ranges)
    scnt_max = max(sb_scnt)

    with ExitStack() as ctx:
        tc = ctx.enter_context(tile.TileContext(nc))
        const = ctx.enter_context(tc.tile_pool(name="const", bufs=1))
        work = ctx.enter_context(tc.tile_pool(name="work", bufs=3))
        prep = ctx.enter_context(tc.tile_pool(name="prep", bufs=2))

        # ---- constants -------------------------------------------------
        t_x0a = const.tile([C, SHARD], bf16)
        nc.sync.dma_start(out=t_x0a[:], in_=d_x0t.ap())
        nc.vector.tensor_scalar_mul(t_x0a[:], t_x0a[:], ALPHA)

        t_w1 = const.tile([C, C], f32)
        nc.sync.dma_start(out=t_w1[:], in_=d_w1.ap())
        t_iden64 = const.tile([C, C], f32)
        nc.sync.dma_start(out=t_iden64[:], in_=d_iden64.ap())

        # w1p = (1-beta) * I + beta * W1  -> bf16 (lhsT of the update matmul)
        t_w1b = const.tile([C, C], f32)
        nc.vector.tensor_scalar_mul(t_w1b[:], t_w1[:], BETA)
        t_idb = const.tile([C, C], f32)
        nc.vector.tensor_scalar_mul(t_idb[:], t_iden64[:], 1.0 - BETA)
        t_w1p = const.tile([C, C], f32)
        nc.vector.tensor_add(t_w1p[:], t_w1b[:], t_idb[:])
        t_w1pb = const.tile([C, C], bf16)
        nc.vector.tensor_copy(t_w1pb[:], t_w1p[:])

        t_h = const.tile([C, SHARD], bf16)   # h (channel-major, bf16)

        # per-superblock weighted segment matrices (prepped on device)
        t_bw = [const.tile([128, sb_col_ranges[s][1] - sb_col_ranges[s][0]],
                           bf16, name=f"bw{s}",
                           padded_shape=[128, sb_cmax]) for s in range(NSB)]

        # ---- main aggregation + per-superblock update -------------------
        npos = 128 * SB_QB                       # positions per superblock
        with tc.tile_pool(name="psum_agg", bufs=2, space="PSUM") as psum_agg, \
             tc.tile_pool(name="psum_o", bufs=2, space="PSUM") as psum_o:
            for sb in range(NSB):
                c_lo, c_hi = sb_col_ranges[sb]
                s_lo, s_hi = sb_ranges[sb]
                # B_w[k, m] = (1-a) * degprod^-1/2 (non-members: 3e38 -> ~0)
                t_bp = prep.tile([128, c_hi - c_lo], bf16, tag="bp",
                                 name=f"bp{sb}", padded_shape=[128, sb_cmax])
                nc.sync.dma_start(out=t_bp[:], in_=d_bp.ap()[:, c_lo:c_hi])
                t_pf = prep.tile([128, c_hi - c_lo], f32, tag="pf",
                                 name=f"pf{sb}", padded_shape=[128, sb_cmax])
                nc.vector.reciprocal(t_pf[:], t_bp[:])
                nc.scalar.activation(
                    t_bw[sb][:], t_pf[:], mybir.ActivationFunctionType.Sqrt,
                    scale=(1.0 - ALPHA) ** 2,
                )

                t_feat = work.tile([128, s_hi - s_lo, 128], bf16, tag="feat",
                                   name=f"feat{sb}",
                                   padded_shape=[128, scnt_max, 128])
                nc.sync.dma_start(out=t_feat[:], in_=d_stream.ap()[:, s_lo:s_hi])

                p_agg = psum_agg.tile([128, 256 * SB_QB], f32, tag="aggblk",
                                      name=f"agg{sb}")
                for si in range(s_lo, s_hi):
                    q0, M, st, sp, _, bcol = slot_meta[si]
                    b = (q0 // QBLK) % SB_QB
                    p0 = q0 % QBLK
                    o0 = b * 256 + 2 * p0
                    nc.tensor.matmul(
                        out=p_agg[:, o0:o0 + 2 * M],
                        lhsT=t_feat[:, si - s_lo],
                        rhs=t_bw[sb][:, bcol:bcol + 2 * M],
                        start=st,
                        stop=sp,
                    )
                # h = agg + alpha*x0 (A: rows 0:64 even cols; B: rows 64:128
                # odd cols); t_h columns [sb*2*npos, +npos) = A, [+npos) = B
                a0 = sb * 2 * npos
                nc.vector.tensor_add(
                    out=t_h[:, a0:a0 + npos],
                    in0=p_agg[0:C, 0:2 * npos:2],
                    in1=t_x0a[:, a0:a0 + npos],
                )
                nc.vector.tensor_add(
                    out=t_h[:, a0 + npos:a0 + 2 * npos],
                    in0=p_agg[C:128, 1:2 * npos:2],
                    in1=t_x0a[:, a0 + npos:a0 + 2 * npos],
                )
                # out = ((1-b) I + b W1)^T @ h for this superblock's 2*npos
                t_oc = work.tile([C, 2 * npos], bf16, tag="ochunk",
                                 name=f"oc{sb}")
                for j in range(2 * npos // 512):
                    p_o = psum_o.tile([C, 512], f32, tag="otile",
                                      name=f"ot{sb}_{j}")
                    nc.tensor.matmul(
                        out=p_o[:],
                        lhsT=t_w1pb[:],
                        rhs=t_h[:, a0 + j * 512:a0 + (j + 1) * 512],
                        start=True,
                        stop=True,
                    )
                    nc.vector.tensor_copy(
                        out=t_oc[:, j * 512:(j + 1) * 512], in_=p_o[:])
                nc.sync.dma_start(
                    out=d_out.ap()[:, a0:a0 + 2 * npos], in_=t_oc[:])

    nc.compile()
    return nc


# --------------------------------------------------------------------------
# entry point
# --------------------------------------------------------------------------

def kernel(x, x_0, weight1, edge_index):
    global LAST_RESULT
    x = np.asarray(x, dtype=np.float32)
    x_0 = np.asarray(x_0, dtype=np.float32)
    weight1 = np.asarray(weight1, dtype=np.float32)
    edge_index = np.asarray(edge_index)

    deg, src_s, node_start, gorder, profile = _prep(edge_index)
    (slot_meta, sb_ranges, sb_col_ranges, lane_pos, lane_colg, lane_eoff,
     ns, SM) = _schedule(profile)
    nc = _build(ns, SM, slot_meta, sb_ranges, sb_col_ranges)

    iden64 = np.eye(C, dtype=np.float32)
    xbf = x.astype(ml_dtypes.bfloat16)
    x0bf = x_0.astype(ml_dtypes.bfloat16)

    li, ki = np.nonzero(lane_pos >= 0)
    pos = lane_pos[li, ki]
    eoff = lane_eoff[li, ki]
    colA = lane_colg[li, ki]

    # position -> output-column map: per superblock, 512 A cols then 512 B
    npos = QBLK * SB_QB

    def _ids_for(gn):
        A, B = gn[0::2], gn[1::2]
        return np.concatenate([
            np.concatenate([A[s * npos:(s + 1) * npos],
                            B[s * npos:(s + 1) * npos]])
            for s in range(NSB)
        ])

    in_maps = []
    for c in range(N_CORES):
        gn = gorder[c::N_CORES]                    # degree-sorted core nodes
        ids = _ids_for(gn)

        stream = np.zeros((128, ns, 128), dtype=ml_dtypes.bfloat16)
        bp = np.full((128, SM), 3.0e38, dtype=ml_dtypes.bfloat16)
        for half, (voff, coff) in enumerate([(0, 0), (1, 1)]):
            v = gn[2 * pos + voff]
            dv = deg[v]
            real = eoff < dv
            e = np.where(real, node_start[v] + eoff, 0)
            gr = src_s[e]
            feats = xbf[gr]
            feats[~real] = 0
            stream[ki, li, half * C:(half + 1) * C] = feats
            bp[ki, colA + coff] = np.where(
                real,
                (deg[gr] * dv).astype(ml_dtypes.bfloat16),
                ml_dtypes.bfloat16(3.0e38))

        x0t = np.ascontiguousarray(x0bf[ids].T)
        in_maps.append({
            "stream": stream,
            "bp": bp,
            "x0t": x0t,
            "w1": weight1,
            "iden64": iden64,
        })

    res = bass_utils.run_bass_kernel_spmd(
        nc, in_maps, core_ids=list(range(N_CORES)),
        trace=bool(os.environ.get("GCN_TRACE")),
    )
    LAST_RESULT = res

    out = np.empty((N_NODES, C), dtype=np.float32)
    for c in range(N_CORES):
        gn = gorder[c::N_CORES]
        ids = _ids_for(gn)
        o = res.results[c]["out"]                  # [C, SHARD] position-major
        out[ids] = o.T.astype(np.float32)
    return out


# revision 17
# speedup vs baseline: 1.3042x; 1.3042x over previous
"""GCN2 (GCNII) aggregation + update kernel for 8 Trainium2 NeuronCores.

Sharding: nodes are assigned to cores by striding the global degree-sorted
order (core c gets ranks c, c+8, ...), so every core sees a near-identical
degree profile and one compiled schedule serves all 8 cores with minimal
padding.  Edges are partitioned by destination; per-edge source rows are
halo-materialized host-side in destination-schedule order (bf16) so the
device streams them sequentially at full DMA bandwidth instead of doing
random 256B gathers.

Within a core, paired destination positions (adjacent degree-sorted ranks)
share each 128-lane slot: the slot's stationary operand is [128 lanes, 128]
with the A-instance features in columns 0:64 and the B-instance features in
columns 64:128, so the full 128x128 PE stationary is used.  The moving
operand interleaves one weighted 0/1 column per instance (A at even, B at
odd columns); output rows 0:64 of even columns carry the A aggregate and
rows 64:128 of odd columns the B aggregate (the complementary halves are
ignored garbage).  Per-edge weights deg(src)^-1/2 * deg(dst)^-1/2 * (1-a)
are computed on device from a bf16 degree-product tensor via Ln/Exp;
non-member and pad entries hold 1e30 so their weight underflows to ~0
(1e-15) without any masking ops.  The alpha
residual with x_0 and the (1-beta)I + beta*W1 update run on device.

Host-side work is strictly structural / data rearrangement: appending
self-loops, bincount, sorting, padding, packing, row duplication and dtype
conversion of x.  No floating-point arithmetic is done on the host.
"""
import math
import os
from contextlib import ExitStack

import numpy as np
import ml_dtypes

import concourse.bacc as bacc
import concourse.mybir as mybir
import concourse.tile as tile
from concourse import bass_utils

N_NODES = 65536
C = 64
N_CORES = 8
SHARD = N_NODES // N_CORES          # 8192 dst nodes per core
NPAIR = SHARD // 2                  # 4096 paired positions per core
QBLK = 128                          # positions per psum block
SB_QB = 4                           # q-blocks per superblock
NQB = NPAIR // QBLK                 # 32
NSB = NQB // SB_QB                  # 8 superblocks
ALPHA = 0.1
BETA = math.log(0.5 / 4 + 1.0)

LAST_RESULT = None  # BassKernelResults of the most recent run (for test.py)


# --------------------------------------------------------------------------
# host-side structural prep (no float math)
# --------------------------------------------------------------------------

def _schedule(profile):
    """Greedy slot schedule over paired positions against `profile` (the
    cross-core max of per-pair degree).  Slots never cross a 128-position
    q-block boundary.  Columns are interleaved (A at even, B at odd) and
    numbered locally per superblock."""
    slots = []                       # (q0, M, start, stop, splits, ebase)
    i = 0
    while i < NPAIR:
        dm = int(profile[i])
        if dm > 128:
            q = (dm + 127) // 128
            for j in range(q):
                lanes = min(128, dm - j * 128)
                slots.append((i, 1, j == 0, j == q - 1, [lanes], j * 128))
            i += 1
        else:
            M = 0
            lanes = 0
            splits = []
            while (
                i + M < NPAIR
                and (i % QBLK) + M < QBLK
                and int(profile[i + M]) <= 128 - lanes
            ):
                splits.append(int(profile[i + M]))
                lanes += int(profile[i + M])
                M += 1
            slots.append((i, M, True, True, splits, 0))
            i += M

    ns = len(slots)
    lane_pos = np.full((ns, 128), -1, dtype=np.int64)
    lane_colg = np.full((ns, 128), -1, dtype=np.int64)  # global A-column
    lane_eoff = np.zeros((ns, 128), dtype=np.int64)
    slot_meta = []                   # (q0, M, start, stop, sb, bcol_local)
    sb_ranges = [[None, None] for _ in range(NSB)]
    sb_cols = [0] * NSB
    for si, (q0, M, st, sp, splits, ebase) in enumerate(slots):
        sb = q0 // (QBLK * SB_QB)
        if sb_ranges[sb][0] is None:
            sb_ranges[sb][0] = si
        sb_ranges[sb][1] = si + 1
        bcol = sb_cols[sb]
        lane = 0
        for m, dmx in enumerate(splits):
            lane_pos[si, lane:lane + dmx] = q0 + m
            lane_colg[si, lane:lane + dmx] = bcol + 2 * m  # local; fixed below
            lane_eoff[si, lane:lane + dmx] = ebase + np.arange(dmx)
            lane += dmx
        slot_meta.append((q0, M, st, sp, sb, bcol))
        sb_cols[sb] += 2 * M
    # per-superblock global column bases
    sb_base = np.zeros(NSB + 1, dtype=np.int64)
    np.cumsum(sb_cols, out=sb_base[1:])
    for si, (q0, M, st, sp, sb, bcol) in enumerate(slot_meta):
        mask = lane_colg[si] >= 0
        lane_colg[si, mask] += sb_base[sb]
    SM = int(sb_base[-1])
    sb_col_ranges = [(int(sb_base[s]), int(sb_base[s + 1])) for s in range(NSB)]
    return slot_meta, [tuple(r) for r in sb_ranges], sb_col_ranges, \
        lane_pos, lane_colg, lane_eoff, ns, SM


def _prep(edge_index: np.ndarray):
    src = np.concatenate([edge_index[0], np.arange(N_NODES, dtype=np.int64)])
    dst = np.concatenate([edge_index[1], np.arange(N_NODES, dtype=np.int64)])
    deg = np.bincount(dst, minlength=N_NODES).astype(np.int64)  # incl self-loops
    assert int(deg.max()) ** 2 < 32768

    order = np.argsort(dst, kind="stable")
    src_s = src[order]
    node_start = np.zeros(N_NODES + 1, dtype=np.int64)
    np.cumsum(deg, out=node_start[1:])

    gorder = np.argsort(-deg, kind="stable")       # global degree-sorted nodes
    gdeg = deg[gorder]
    # core c owns gorder[c::8]; pair q = local ranks (2q, 2q+1)
    # profile[q] = max over cores of deg at local rank 2q = gdeg[16q]
    profile = gdeg[0::2 * N_CORES].copy()          # [NPAIR]
    return deg, src_s, node_start, gorder, profile


# --------------------------------------------------------------------------
# device kernel
# --------------------------------------------------------------------------

def _build(ns, SM, slot_meta, sb_ranges, sb_col_ranges):
    f32, bf16, i16 = mybir.dt.float32, mybir.dt.bfloat16, mybir.dt.int16
    nc = bacc.Bacc("TRN2", debug=False, num_devices=N_CORES)

    d_stream = nc.dram_tensor("stream", [128, ns, 128], bf16, kind="ExternalInput")
    d_bp = nc.dram_tensor("bp", [128, SM], bf16, kind="ExternalInput")
    d_x0t = nc.dram_tensor("x0t", [C, SHARD], bf16, kind="ExternalInput")
    d_w1 = nc.dram_tensor("w1", [C, C], f32, kind="ExternalInput")
    d_iden64 = nc.dram_tensor("iden64", [C, C], f32, kind="ExternalInput")
    d_out = nc.dram_tensor("out", [C, SHARD], bf16, kind="ExternalOutput")

    sb_scnt = [hi - lo for lo, hi in sb_ranges]
    sb_cmax = max(hi - lo for lo, hi in sb_col_ranges)
    scnt_max = max(sb_scnt)

    with ExitStack() as ctx:
        tc = ctx.enter_context(tile.TileContext(nc))
        const = ctx.enter_context(tc.tile_pool(name="const", bufs=1))
        work = ctx.enter_context(tc.tile_pool(name="work", bufs=3))
        prep = ctx.enter_context(tc.tile_pool(name="prep", bufs=2))

        # ---- constants -------------------------------------------------
        t_x0a = const.tile([C, SHARD], bf16)
        nc.sync.dma_start(out=t_x0a[:], in_=d_x0t.ap())
        nc.vector.tensor_scalar_mul(t_x0a[:], t_x0a[:], ALPHA)

        t_w1 = const.tile([C, C], f32)
        nc.sync.dma_start(out=t_w1[:], in_=d_w1.ap())
        t_iden64 = const.tile([C, C], f32)
        nc.sync.dma_start(out=t_iden64[:], in_=d_iden64.ap())

        # w1p = (1-beta) * I + beta * W1  -> bf16 (lhsT of the update matmul)
        t_w1b = const.tile([C, C], f32)
        nc.vector.tensor_scalar_mul(t_w1b[:], t_w1[:], BETA)
        t_idb = const.tile([C, C], f32)
        nc.vector.tensor_scalar_mul(t_idb[:], t_iden64[:], 1.0 - BETA)
        t_w1p = const.tile([C, C], f32)
        nc.vector.tensor_add(t_w1p[:], t_w1b[:], t_idb[:])
        t_w1pb = const.tile([C, C], bf16)
        nc.vector.tensor_copy(t_w1pb[:], t_w1p[:])

        t_h = const.tile([C, SHARD], bf16)   # h (channel-major, bf16)

        # per-superblock weighted segment matrices (prepped on device)
        t_bw = [const.tile([128, sb_col_ranges[s][1] - sb_col_ranges[s][0]],
                           bf16, name=f"bw{s}",
                           padded_shape=[128, sb_cmax]) for s in range(NSB)]

        # ---- main aggregation + per-superblock update -------------------
        npos = 128 * SB_QB                       # positions per superblock
        with tc.tile_pool(name="psum_agg", bufs=3, space="PSUM") as psum_agg, \
             tc.tile_pool(name="psum_o", bufs=2, space="PSUM") as psum_o:
            for sb in range(NSB):
                c_lo, c_hi = sb_col_ranges[sb]
                s_lo, s_hi = sb_ranges[sb]
                # B_w[k, m] = (1-a) * degprod^-1/2 (non-members: 3e38 -> ~0)
                t_bp = prep.tile([128, c_hi - c_lo], bf16, tag="bp",
                                 name=f"bp{sb}", padded_shape=[128, sb_cmax])
                nc.sync.dma_start(out=t_bp[:], in_=d_bp.ap()[:, c_lo:c_hi])
                t_pc = prep.tile([128, c_hi - c_lo], f32, tag="pc",
                                 name=f"pc{sb}", padded_shape=[128, sb_cmax])
                nc.gpsimd.tensor_copy(t_pc[:], t_bp[:])
                t_pf = prep.tile([128, c_hi - c_lo], f32, tag="pf",
                                 name=f"pf{sb}", padded_shape=[128, sb_cmax])
                nc.vector.reciprocal_approx_fast(t_pf[:], t_pc[:])
                nc.scalar.activation(
                    t_bw[sb][:], t_pf[:], mybir.ActivationFunctionType.Sqrt,
                    scale=(1.0 - ALPHA) ** 2,
                )

                t_feat = work.tile([128, s_hi - s_lo, 128], bf16, tag="feat",
                                   name=f"feat{sb}",
                                   padded_shape=[128, scnt_max, 128])
                nc.sync.dma_start(out=t_feat[:], in_=d_stream.ap()[:, s_lo:s_hi])

                p_agg = psum_agg.tile([128, 256 * SB_QB], f32, tag="aggblk",
                                      name=f"agg{sb}")
                for si in range(s_lo, s_hi):
                    q0, M, st, sp, _, bcol = slot_meta[si]
                    b = (q0 // QBLK) % SB_QB
                    p0 = q0 % QBLK
                    o0 = b * 256 + 2 * p0
                    nc.tensor.matmul(
                        out=p_agg[:, o0:o0 + 2 * M],
                        lhsT=t_feat[:, si - s_lo],
                        rhs=t_bw[sb][:, bcol:bcol + 2 * M],
                        start=st,
                        stop=sp,
                    )
                # h = agg + alpha*x0 (A: rows 0:64 even cols; B: rows 64:128
                # odd cols); t_h columns [sb*2*npos, +npos) = A, [+npos) = B
                a0 = sb * 2 * npos
                nc.vector.tensor_add(
                    out=t_h[:, a0:a0 + npos],
                    in0=p_agg[0:C, 0:2 * npos:2],
                    in1=t_x0a[:, a0:a0 + npos],
                )
                nc.vector.tensor_add(
                    out=t_h[:, a0 + npos:a0 + 2 * npos],
                    in0=p_agg[C:128, 1:2 * npos:2],
                    in1=t_x0a[:, a0 + npos:a0 + 2 * npos],
                )
                # out = ((1-b) I + b W1)^T @ h for this superblock's 2*npos
                t_oc = work.tile([C, 2 * npos], bf16, tag="ochunk",
                                 name=f"oc{sb}")
                for j in range(2 * npos // 512):
                    p_o = psum_o.tile([C, 512], f32, tag="otile",
                                      name=f"ot{sb}_{j}")
                    nc.tensor.matmul(
                        out=p_o[:],
                        lhsT=t_w1pb[:],
                        rhs=t_h[:, a0 + j * 512:a0 + (j + 1) * 512],
                        start=True,
                        stop=True,
                    )
                    nc.vector.tensor_copy(
                        out=t_oc[:, j * 512:(j + 1) * 512], in_=p_o[:])
                nc.sync.dma_start(
                    out=d_out.ap()[:, a0:a0 + 2 * npos], in_=t_oc[:])

    nc.compile()
    return nc


# --------------------------------------------------------------------------
# entry point
# --------------------------------------------------------------------------

def kernel(x, x_0, weight1, edge_index):
    global LAST_RESULT
    x = np.asarray(x, dtype=np.float32)
    x_0 = np.asarray(x_0, dtype=np.float32)
    weight1 = np.asarray(weight1, dtype=np.float32)
    edge_index = np.asarray(edge_index)

    deg, src_s, node_start, gorder, profile = _prep(edge_index)
    (slot_meta, sb_ranges, sb_col_ranges, lane_pos, lane_colg, lane_eoff,
     ns, SM) = _schedule(profile)
    nc = _build(ns, SM, slot_meta, sb_ranges, sb_col_ranges)

    iden64 = np.eye(C, dtype=np.float32)
    xbf = x.astype(ml_dtypes.bfloat16)
    x0bf = x_0.astype(ml_dtypes.bfloat16)

    li, ki = np.nonzero(lane_pos >= 0)
    pos = lane_pos[li, ki]
    eoff = lane_eoff[li, ki]
    colA = lane_colg[li, ki]

    # position -> output-column map: per superblock, 512 A cols then 512 B
    npos = QBLK * SB_QB

    def _ids_for(gn):
        A, B = gn[0::2], gn[1::2]
        return np.concatenate([
            np.concatenate([A[s * npos:(s + 1) * npos],
                            B[s * npos:(s + 1) * npos]])
            for s in range(NSB)
        ])

    in_maps = []
    for c in range(N_CORES):
        gn = gorder[c::N_CORES]                    # degree-sorted core nodes
        ids = _ids_for(gn)

        stream = np.zeros((128, ns, 128), dtype=ml_dtypes.bfloat16)
        bp = np.full((128, SM), 1.0e30, dtype=ml_dtypes.bfloat16)
        for half, (voff, coff) in enumerate([(0, 0), (1, 1)]):
            v = gn[2 * pos + voff]
            dv = deg[v]
            real = eoff < dv
            e = np.where(real, node_start[v] + eoff, 0)
            gr = src_s[e]
            feats = xbf[gr]
            feats[~real] = 0
            stream[ki, li, half * C:(half + 1) * C] = feats
            bp[ki, colA + coff] = np.where(
                real,
                (deg[gr] * dv).astype(ml_dtypes.bfloat16),
                ml_dtypes.bfloat16(1.0e30))

        x0t = np.ascontiguousarray(x0bf[ids].T)
        in_maps.append({
            "stream": stream,
            "bp": bp,
            "x0t": x0t,
            "w1": weight1,
            "iden64": iden64,
        })

    res = bass_utils.run_bass_kernel_spmd(
        nc, in_maps, core_ids=list(range(N_CORES)),
        trace=bool(os.environ.get("GCN_TRACE")),
    )
    LAST_RESULT = res

    out = np.empty((N_NODES, C), dtype=np.float32)
    for c in range(N_CORES):
        gn = gorder[c::N_CORES]
        ids = _ids_for(gn)
        o = res.results[c]["out"]                  # [C, SHARD] position-major
        out[ids] = o.T.astype(np.float32)
    return out
